# revision 10
# baseline (speedup 1.0000x reference)
"""Trainium2 Bass kernel for nn_ModAttn (modulated multi-function attention).

Shapes: x [1,1024,512], compatibility [1,4,1024]; out [1,4,1024,512].

Sharding: 8 cores = (function f in 0..3) x (head-half hh in 0..1). Each core
computes 4 of the 8 heads for its function over ALL 1024 queries/keys, then
projects its 256 ym-dims through the matching W_proj rows; the host sums the
two partial projections per function (each core adds b_proj/2 so the pair
sums to b_proj). No k/v duplication and no collectives.

Attention runs with QUERIES on the partition axis ([n, m] orientation):
  exp1 = exp(scale*S) per 128-query chunk with accum_out -> s (softmax-1 sums
  are free), t2 = (e1 * 1/s) * C via DVE 4x tensor_scalar + 2x tensor_tensor,
  e2 = exp(t2) in two big in-place ACT passes, then e2 is flipped to [m, n]
  with XBAR DMA transposes (idle DMA engines) for the PV matmul. A ones
  column in v yields z2 (softmax-2 sums) as PV row 64; 1/z2 is broadcast by
  a PE ones-matmul and folded into the PSUM->SBUF move of ym. cm_p is folded
  into W_proj rows once.

Per-core hh-dependence is carried entirely by data: the host permutes the
din axis (x^T rows, w_c^T cols, ln_qkv params, W_qkv^T rows) so each core's
256 proj-input dims come first in its layernorm modulation vector.
"""

import numpy as np
from contextlib import ExitStack

import ml_dtypes

N_CORES = 8
N, DIN, NF, H = 1024, 512, 4, 8
HH = H // 2          # heads per core
HD = DIN // H        # 64
HDIM = HH * HD       # 256 dims per core
SCALE = HD ** -0.5

_CACHE = {}


def build_nc():
    import concourse.bacc as bacc
    import concourse.tile as tile
    from concourse import mybir
    from concourse.masks import make_identity

    F32 = mybir.dt.float32
    F32R = mybir.dt.float32r
    BF16 = mybir.dt.bfloat16
    AT = mybir.ActivationFunctionType
    OP = mybir.AluOpType

    nc = bacc.Bacc("TRN2", target_bir_lowering=False, debug=False,
                   num_devices=N_CORES)

    xtb_d = nc.dram_tensor("xtb", [DIN, N], BF16, kind="ExternalInput")
    wqk_d = nc.dram_tensor("wqk", [DIN, 2 * HDIM], BF16, kind="ExternalInput")
    wv_d = nc.dram_tensor("wv", [DIN, HDIM], BF16, kind="ExternalInput")
    wp_d = nc.dram_tensor("wp", [HDIM, DIN], BF16, kind="ExternalInput")
    comp_d = nc.dram_tensor("comp", [NF, N], BF16, kind="ExternalInput")
    codef_d = nc.dram_tensor("codef", [128, 1], F32, kind="ExternalInput")
    wct_d = nc.dram_tensor("wct", [128, DIN], F32, kind="ExternalInput")
    bqk_d = nc.dram_tensor("bqk", [2 * HDIM], F32, kind="ExternalInput")
    bv_d = nc.dram_tensor("bv", [1, HDIM], F32, kind="ExternalInput")
    bp_d = nc.dram_tensor("bp", [1, DIN], F32, kind="ExternalInput")
    lnqg_d = nc.dram_tensor("lnqg", [1, DIN], F32, kind="ExternalInput")
    lnqb_d = nc.dram_tensor("lnqb", [1, DIN], F32, kind="ExternalInput")
    lnpg_d = nc.dram_tensor("lnpg", [1, HDIM], F32, kind="ExternalInput")
    lnpb_d = nc.dram_tensor("lnpb", [1, HDIM], F32, kind="ExternalInput")
    y_d = nc.dram_tensor("y", [N, DIN], F32, kind="ExternalOutput")

    with tile.TileContext(nc) as tc, ExitStack() as top:
        const = top.enter_context(tc.tile_pool(name="const", bufs=1))
        ones_r = const.tile([1, 128], F32, tag="ones_r")
        nc.vector.memset(ones_r[:], 1.0)
        ones_rb = const.tile([1, 128], BF16, tag="ones_rb")
        nc.vector.memset(ones_rb[:], 1.0)
        cmT_q = const.tile([128, 4], F32, tag="cmT_q")
        cmT_p = const.tile([128, 2], F32, tag="cmT_p")

        # big input loads first so HBM transfers overlap the setup chain
        big = top.enter_context(tc.tile_pool(name="big", bufs=1))
        xt = [big.tile([128, N], BF16, tag=f"xt{c}", name=f"xt{c}")
              for c in range(4)]
        wqk = [big.tile([128, 2 * HDIM], BF16, tag=f"wqk{c}", name=f"wqk{c}")
               for c in range(4)]
        wv = [big.tile([128, HDIM], BF16, tag=f"wv{c}", name=f"wv{c}")
              for c in range(4)]
        wpm = [big.tile([128, DIN], BF16, tag=f"wpm{c}", name=f"wpm{c}")
               for c in range(2)]
        for c in range(4):
            nc.gpsimd.dma_start(xt[c][:], xtb_d.ap()[c * 128:(c + 1) * 128, :])
        for c in range(4):
            nc.gpsimd.dma_start(wqk[c][:], wqk_d.ap()[c * 128:(c + 1) * 128, :])
        for c in range(4):
            nc.gpsimd.dma_start(wv[c][:], wv_d.ap()[c * 128:(c + 1) * 128, :])
        wp_raw = [big.tile([128, DIN], BF16, tag=f"wpr{c}", name=f"wpr{c}")
                  for c in range(2)]
        for c in range(2):
            nc.gpsimd.dma_start(wp_raw[c][:], wp_d.ap()[c * 128:(c + 1) * 128, :])

        # ---------- phase A: modulation vectors ----------
        with tc.tile_pool(name="smA", bufs=1) as smA, \
             tc.tile_pool(name="psA", bufs=1, space="PSUM") as psA:
            wct_t = smA.tile([128, DIN], F32, tag="wct")
            nc.sync.dma_start(wct_t[:], wct_d.ap())
            codef_t = smA.tile([128, 1], F32, tag="codef")
            nc.sync.dma_start(codef_t[:], codef_d.ap())
            lnt = {}
            for nm, d in (("qg", lnqg_d), ("qb", lnqb_d)):
                lnt[nm] = smA.tile([1, DIN], F32, tag=f"ln{nm}", name=f"ln{nm}")
                nc.sync.dma_start(lnt[nm][:], d.ap())
            for nm, d in (("pg", lnpg_d), ("pb", lnpb_d)):
                lnt[nm] = smA.tile([1, HDIM], F32, tag=f"ln{nm}", name=f"ln{nm}")
                nc.sync.dma_start(lnt[nm][:], d.ap())

            cm0_ps = psA.tile([1, DIN], F32, tag="cm0ps")
            nc.tensor.matmul(cm0_ps[:], codef_t[:], wct_t[:], start=True,
                             stop=True)
            cm0 = smA.tile([1, DIN], F32, tag="cm0")
            nc.vector.tensor_copy(cm0[:], cm0_ps[:])

            # shared LN stats for both param sets (same cm0; din-permutation
            # of the host layout leaves mean/var invariant)
            st = smA.tile([1, 1], F32, tag="st")
            nc.vector.tensor_reduce(st[:], cm0[:], mybir.AxisListType.X, OP.add)
            mu = smA.tile([1, 1], F32, tag="mu")
            nc.vector.tensor_scalar_mul(mu[:], st[:], 1.0 / DIN)
            sq = smA.tile([1, DIN], F32, tag="sq")
            vacc = smA.tile([1, 1], F32, tag="vacc")
            nc.vector.scalar_tensor_tensor(sq[:], cm0[:], mu[:], cm0[:],
                                           OP.subtract, OP.mult,
                                           accum_out=vacc[:])
            ve = smA.tile([1, 1], F32, tag="ve")
            nc.vector.tensor_scalar(ve[:], vacc[:], 1.0 / DIN, 1e-5,
                                    OP.mult, OP.add)
            sd = smA.tile([1, 1], F32, tag="sd")
            nc.scalar.activation(sd[:], ve[:], AT.Sqrt)
            rstd = smA.tile([1, 1], F32, tag="rstd")
            nc.vector.reciprocal(rstd[:], sd[:])

            def layer_norm(pref, g, b, dim):
                rg = smA.tile([1, dim], F32, tag=f"{pref}rg", name=f"{pref}rg")
                nc.vector.tensor_scalar_mul(rg[:], g[:], rstd[:])
                cx = smA.tile([1, dim], F32, tag=f"{pref}cx", name=f"{pref}cx")
                nc.vector.scalar_tensor_tensor(cx[:], cm0[:, 0:dim], mu[:],
                                               rg[:], OP.subtract, OP.mult)
                cm = smA.tile([1, dim], F32, tag=f"{pref}cm", name=f"{pref}cm")
                nc.vector.tensor_add(cm[:], cx[:], b[:])
                return cm

            cmq = layer_norm("q", lnt["qg"], lnt["qb"], DIN)
            cmp_ = layer_norm("p", lnt["pg"], lnt["pb"], HDIM)
            for c in range(4):
                tp = psA.tile([128, 1], F32, tag="cmtp", name="cmtp")
                nc.tensor.transpose(tp[:], cmq[:, c * 128:(c + 1) * 128],
                                    ones_r[0:1, 0:1])
                nc.vector.tensor_copy(cmT_q[:, c:c + 1], tp[:])
            for c in range(2):
                tp = psA.tile([128, 1], F32, tag="cmtp", name="cmtp")
                nc.tensor.transpose(tp[:], cmp_[:, c * 128:(c + 1) * 128],
                                    ones_r[0:1, 0:1])
                nc.vector.tensor_copy(cmT_p[:, c:c + 1], tp[:])
        # fold cm_p into the W_proj rows (proj-input dims on partitions)
        for c in range(2):
            nc.vector.tensor_scalar_mul(wpm[c][:], wp_raw[c][:],
                                        cmT_p[:, c:c + 1])

        # ---------- persistent attention operands ----------
        qkv = top.enter_context(tc.tile_pool(name="qkv", bufs=1))
        qkT = [qkv.tile([128, N], F32R, tag=f"qkT{j}", name=f"qkT{j}")
               for j in range(4)]  # j 0,1 = q head-pairs; 2,3 = k head-pairs
        vv = [qkv.tile([128, HH * (HD + 1)], BF16, tag=f"vv{m}", name=f"vv{m}")
              for m in range(8)]
        Ct = qkv.tile([128, 8 * N], BF16, tag="Ct")  # C[nc*128+p, m]
        ymT = [qkv.tile([128, N], BF16, tag=f"ymT{c}", name=f"ymT{c}")
               for c in range(2)]

        # ---------- phase C: compatibility outer product ----------
        with tc.tile_pool(name="smC", bufs=1) as smC, \
             tc.tile_pool(name="psC", bufs=2, space="PSUM") as psC:
            comp_r = smC.tile([NF, N], BF16, tag="comp_r")
            nc.sync.dma_start(comp_r[:], comp_d.ap())
            for nch in range(8):
                ps = psC.tile([128, N], F32, tag="psc", name="psc")
                for half in range(2):
                    nc.tensor.matmul(ps[:, half * 512:(half + 1) * 512],
                                     comp_r[:, nch * 128:(nch + 1) * 128],
                                     comp_r[:, half * 512:(half + 1) * 512],
                                     start=True, stop=True)
                # PSUM->SBUF moves split across ACT/DVE; both are idle this
                # early (the exp stream starts only after QKV+scores)
                if nch % 2 == 0:
                    nc.vector.tensor_copy(Ct[:, nch * N:(nch + 1) * N], ps[:])
                else:
                    nc.scalar.copy(Ct[:, nch * N:(nch + 1) * N], ps[:])

        # ---------- phase B: QKV projections ----------
        with tc.tile_pool(name="smB", bufs=1) as smB, \
             tc.tile_pool(name="psB", bufs=2, space="PSUM") as psB, \
             tc.tile_pool(name="psV", bufs=2, space="PSUM") as psV:
            xm = [smB.tile([128, N], BF16, tag=f"xm{c}", name=f"xm{c}")
                  for c in range(4)]
            for c in range(4):
                nc.vector.tensor_scalar_mul(xm[c][:], xt[c][:],
                                            cmT_q[:, c:c + 1])
            bqk_t = smB.tile([128, 4], F32, tag="bqk")
            for j in range(4):
                nc.sync.dma_start(bqk_t[:, j:j + 1],
                                  bqk_d.ap()[j * 128:(j + 1) * 128])
            bv_raw = smB.tile([1, HDIM], F32, tag="bv_raw")
            nc.sync.dma_start(bv_raw[:], bv_d.ap())
            bvb = smB.tile([128, HDIM], F32, tag="bvb")
            nc.gpsimd.partition_broadcast(bvb[:], bv_raw[:], channels=128)

            # q then k, emitting the head-pair tiles scores need first
            for j in (0, 2, 1, 3):
                ps = psB.tile([128, N], F32, tag="psb", name="psb")
                for half in range(2):
                    for c in range(4):
                        nc.tensor.matmul(
                            ps[:, half * 512:(half + 1) * 512],
                            wqk[c][:, j * 128:(j + 1) * 128],
                            xm[c][:, half * 512:(half + 1) * 512],
                            start=(c == 0), stop=(c == 3))
                nc.vector.tensor_scalar_add(qkT[j][:], ps[:], bqk_t[:, j:j + 1])
            for m in range(8):  # v natural [128 keys, 4*(64+1)] with ones col
                ps = psV.tile([128, HDIM], F32, tag="psv", name="psv")
                for c in range(4):
                    nc.tensor.matmul(ps[:], xm[c][:, m * 128:(m + 1) * 128],
                                     wv[c][:], start=(c == 0), stop=(c == 3))
                v3 = vv[m][:].rearrange("p (h e) -> p h e", e=HD + 1)
                nc.vector.tensor_add(v3[:, :, 0:HD],
                                     ps[:].rearrange("p (h e) -> p h e", e=HD),
                                     bvb[:].rearrange("p (h e) -> p h e", e=HD))
                nc.vector.memset(v3[:, :, HD:HD + 1], 1.0)

        bp_raw = const.tile([1, DIN], F32, tag="bp_raw")
        nc.sync.dma_start(bp_raw[:], bp_d.ap())
        bp_row = const.tile([1, DIN], BF16, tag="bp_row")
        nc.vector.tensor_copy(bp_row[:], bp_raw[:])

        # ---------- phase D: attention, queries on partitions ----------
        with tc.tile_pool(name="smE1", bufs=2) as smE1, \
             tc.tile_pool(name="smT2", bufs=2) as smT2, \
             tc.tile_pool(name="smTT", bufs=2) as smTT, \
             tc.tile_pool(name="smZ", bufs=2) as smZ, \
             tc.tile_pool(name="smS", bufs=2) as smS, \
             tc.tile_pool(name="psS", bufs=3, space="PSUM") as psS, \
             tc.tile_pool(name="psY", bufs=1, space="PSUM") as psY:
            state = {}

            def emit_d1(h):
                """Scores + exp1(+s) + 1/s + t2 for all 8 query chunks."""
                qj, qo = h // 2, (h % 2) * 64
                e1 = smE1.tile([128, 8 * N], BF16, tag="e1", name="e1")
                t2 = smT2.tile([128, 8 * N], BF16, tag="t2", name="t2")
                srs = smS.tile([128, 16], F32, tag="srs", name="srs")
                for nch in range(8):
                    ps = psS.tile([128, N], F32, tag="ps_s", name="ps_s")
                    for half in range(2):
                        nc.tensor.matmul(
                            ps[:, half * 512:(half + 1) * 512],
                            qkT[qj][qo:qo + 64, nch * 128:(nch + 1) * 128],
                            qkT[2 + qj][qo:qo + 64, half * 512:(half + 1) * 512],
                            start=True, stop=True)
                    nc.scalar.activation(e1[:, nch * N:(nch + 1) * N], ps[:],
                                         AT.Exp, scale=SCALE,
                                         accum_out=srs[:, nch:nch + 1])
                    nc.vector.reciprocal(srs[:, 8 + nch:9 + nch],
                                         srs[:, nch:nch + 1])
                    nc.vector.tensor_scalar_mul(t2[:, nch * N:(nch + 1) * N],
                                                e1[:, nch * N:(nch + 1) * N],
                                                srs[:, 8 + nch:9 + nch])
                    nc.vector.tensor_mul(t2[:, nch * N:(nch + 1) * N],
                                         t2[:, nch * N:(nch + 1) * N],
                                         Ct[:, nch * N:(nch + 1) * N])
                state[h] = (e1, t2)

            def emit_d2(h):
                """exp2 (2 big in-place passes) + XBAR transposes."""
                e1, t2 = state[h]
                e2T = smTT.tile([128, 8 * N], BF16, tag="e2T", name="e2T")
                e2T3 = e2T[:].rearrange("p (mc q) -> p mc q", q=N)
                for ph in range(2):
                    nc.scalar.activation(t2[:, ph * 4 * N:(ph + 1) * 4 * N],
                                         t2[:, ph * 4 * N:(ph + 1) * 4 * N],
                                         AT.Exp)
                    for nch in range(4 * ph, 4 * ph + 4):
                        nc.sync.dma_start_transpose(
                            e2T3[:, :, nch * 128:(nch + 1) * 128],
                            t2[:, nch * N:(nch + 1) * N])
                state[h] = (e2T,)

            def emit_d3(h):
                """PV + z2 normalization into ymT (bf16)."""
                (e2T,) = state.pop(h)
                ypv = psY.tile([HD + 1, N], F32, tag="ypv", name="ypv")
                for mc in range(8):
                    for half in range(2):
                        nc.tensor.matmul(
                            ypv[:, half * 512:(half + 1) * 512],
                            vv[mc][:, h * (HD + 1):(h + 1) * (HD + 1)],
                            e2T[:, mc * N + half * 512:mc * N + (half + 1) * 512],
                            start=(mc == 0), stop=(mc == 7))
                zr = smZ.tile([1, N], BF16, tag="zr", name="zr")
                with nc.allow_low_precision(reason="1/z2 in bf16; z2 is O(1e3)"):
                    nc.vector.reciprocal(zr[:], ypv[HD:HD + 1, :])
                zbs = smZ.tile([64, N], BF16, tag="zbs", name="zbs")
                nc.gpsimd.partition_broadcast(zbs[:], zr[:], channels=64)
                nc.vector.tensor_mul(ymT[h // 2][(h % 2) * 64:(h % 2) * 64 + 64, :],
                                     ypv[0:HD, :], zbs[:])

            emit_d1(0)
            emit_d1(1)
            emit_d2(0)
            emit_d1(2)
            emit_d2(1)
            emit_d3(0)
            emit_d1(3)
            emit_d2(2)
            emit_d3(1)
            emit_d2(3)
            emit_d3(2)
            emit_d3(3)

        # ---------- phase E: output projection (partial, host sums pairs) ----
        with tc.tile_pool(name="smE", bufs=2) as smE, \
             tc.tile_pool(name="psE", bufs=2, space="PSUM") as psE:
            for nb in range(8):
                ps = psE.tile([128, DIN], F32, tag="ps_e")
                for c in range(2):
                    nc.tensor.matmul(ps[:], ymT[c][:, nb * 128:(nb + 1) * 128],
                                     wpm[c][:], start=(c == 0), stop=False)
                nc.tensor.matmul(ps[:], ones_rb[:], bp_row[:], start=False,
                                 stop=True)
                yo = smE.tile([128, DIN], F32, tag="yo")
                nc.vector.tensor_copy(yo[:], ps[:])
                nc.sync.dma_start(y_d.ap()[nb * 128:(nb + 1) * 128, :], yo[:])

    nc.compile()
    return nc


def make_in_maps(x, compatibility, code, w_c, W_qkv, b_qkv, W_proj, b_proj,
                 ln_qkv_g, ln_qkv_b, ln_proj_g, ln_proj_b):
    bf = ml_dtypes.bfloat16
    x = np.asarray(x, np.float32)
    compatibility = np.asarray(compatibility, np.float32)
    code = np.asarray(code, np.float32)
    w_c = np.asarray(w_c, np.float32)
    W_qkv = np.asarray(W_qkv, np.float32)
    b_qkv = np.asarray(b_qkv, np.float32)
    W_proj = np.asarray(W_proj, np.float32)
    b_proj = np.asarray(b_proj, np.float32)
    ln_qkv_g = np.asarray(ln_qkv_g, np.float32)
    ln_qkv_b = np.asarray(ln_qkv_b, np.float32)
    ln_proj_g = np.asarray(ln_proj_g, np.float32)
    ln_proj_b = np.asarray(ln_proj_b, np.float32)

    xT = x[0].T  # [din, n]
    comp_bf = compatibility[0].astype(bf)
    in_maps = []
    for core in range(N_CORES):
        f, hh = core // 2, core % 2
        d0 = hh * HDIM
        # din permutation: this core's proj-input dims first
        perm = np.r_[d0:d0 + HDIM,
                     np.setdiff1d(np.arange(DIN), np.arange(d0, d0 + HDIM))]
        qrows = np.r_[d0:d0 + HDIM]          # q out-dims for heads hh*4..
        krows = np.r_[DIN + d0:DIN + d0 + HDIM]
        vrows = np.r_[2 * DIN + d0:2 * DIN + d0 + HDIM]
        in_maps.append(dict(
            xtb=np.ascontiguousarray(xT[perm, :]).astype(bf),
            wqk=np.ascontiguousarray(
                W_qkv[np.r_[qrows, krows], :][:, perm].T).astype(bf),
            wv=np.ascontiguousarray(W_qkv[vrows, :][:, perm].T).astype(bf),
            wp=np.ascontiguousarray(W_proj.T[d0:d0 + HDIM, :]).astype(bf),
            comp=comp_bf,
            codef=np.ascontiguousarray(code[:, f:f + 1]),
            wct=np.ascontiguousarray(w_c.T[:, perm]),
            bqk=np.ascontiguousarray(b_qkv[np.r_[qrows, krows]]),
            bv=np.ascontiguousarray(b_qkv[vrows]).reshape(1, HDIM),
            bp=(b_proj * 0.5).reshape(1, DIN),
            lnqg=np.ascontiguousarray(ln_qkv_g[perm]).reshape(1, DIN),
            lnqb=np.ascontiguousarray(ln_qkv_b[perm]).reshape(1, DIN),
            lnpg=np.ascontiguousarray(
                ln_proj_g[d0:d0 + HDIM]).reshape(1, HDIM),
            lnpb=np.ascontiguousarray(
                ln_proj_b[d0:d0 + HDIM]).reshape(1, HDIM),
        ))
    return in_maps


def kernel(**inputs) -> np.ndarray:
    from concourse.bass_utils import run_bass_kernel_spmd
    if "nc" not in _CACHE:
        _CACHE["nc"] = build_nc()
    nc = _CACHE["nc"]
    in_maps = make_in_maps(**inputs)
    res = run_bass_kernel_spmd(nc, in_maps, core_ids=list(range(N_CORES)))
    out = np.zeros((1, NF, N, DIN), np.float32)
    for core in range(N_CORES):
        f = core // 2
        out[0, f] += np.asarray(res.results[core]["y"], np.float32)
    return out


# revision 14
# speedup vs baseline: 1.0289x; 1.0289x over previous
"""Trainium2 Bass kernel for nn_ModAttn (modulated multi-function attention).

Shapes: x [1,1024,512], compatibility [1,4,1024]; out [1,4,1024,512].

Sharding: 8 cores = (function f in 0..3) x (head-half hh in 0..1). Each core
computes 4 of the 8 heads for its function over ALL 1024 queries/keys, then
projects its 256 ym-dims through the matching W_proj rows; the host sums the
two partial projections per function (each core adds b_proj/2 so the pair
sums to b_proj). No k/v duplication and no collectives.

Attention runs with QUERIES on the partition axis ([n, m] orientation):
  exp1 = exp(scale*S) per 128-query chunk with accum_out -> s (softmax-1 sums
  are free), t2 = (e1 * 1/s) * C via DVE tensor_scalar + tensor_tensor (all
  bf16), e2 = exp(t2) in two big in-place ACT passes, then e2 is flipped to
  [m, n] with XBAR DMA transposes (idle DMA engines) for the PV matmul. A
  ones column in v yields z2 (softmax-2 sums) as PV row 64; 1/z2 is
  broadcast by GpSimd and folded into the PSUM->SBUF move of ym. cm_p is
  folded into the W_proj rows once.

Emission is software-pipelined so the ACT exp stream (the critical ~75us of
work) starts early and never starves: the C outer-product runs during the
input DMA window, scores for head 0 go right after the first q/k tiles, and
the remaining QKV/v matmuls fill the PE gaps between score batches. PSUM:
psS (2 bufs x [128,1024] = 4 banks, shared by C/qk/scores/proj), psY (ypv,
2 banks), psW (v + phase-A scratch, 2 banks).

Per-core hh-dependence is carried entirely by data: the host permutes the
din axis (x^T rows, w_c^T cols, ln_qkv params, W_qkv^T rows) so each core's
256 proj-input dims come first in its layernorm modulation vector.
"""

import numpy as np
from contextlib import ExitStack

import ml_dtypes

N_CORES = 8
N, DIN, NF, H = 1024, 512, 4, 8
HH = H // 2          # heads per core
HD = DIN // H        # 64
HDIM = HH * HD       # 256 dims per core
SCALE = HD ** -0.5

_CACHE = {}


def build_nc():
    import concourse.bacc as bacc
    import concourse.tile as tile
    from concourse import mybir

    F32 = mybir.dt.float32
    BF16 = mybir.dt.bfloat16
    AT = mybir.ActivationFunctionType
    OP = mybir.AluOpType

    nc = bacc.Bacc("TRN2", target_bir_lowering=False, debug=False,
                   num_devices=N_CORES)

    xtb_d = nc.dram_tensor("xtb", [DIN, N], BF16, kind="ExternalInput")
    wqk_d = nc.dram_tensor("wqk", [DIN, 2 * HDIM], BF16, kind="ExternalInput")
    wv_d = nc.dram_tensor("wv", [DIN, HDIM], BF16, kind="ExternalInput")
    wp_d = nc.dram_tensor("wp", [HDIM, DIN], BF16, kind="ExternalInput")
    comp_d = nc.dram_tensor("comp", [NF, N], BF16, kind="ExternalInput")
    codef_d = nc.dram_tensor("codef", [128, 1], F32, kind="ExternalInput")
    wct_d = nc.dram_tensor("wct", [128, DIN], F32, kind="ExternalInput")
    bqk_d = nc.dram_tensor("bqk", [2 * HDIM], F32, kind="ExternalInput")
    bv_d = nc.dram_tensor("bv", [1, HDIM], F32, kind="ExternalInput")
    bp_d = nc.dram_tensor("bp", [1, DIN], F32, kind="ExternalInput")
    lnqg_d = nc.dram_tensor("lnqg", [1, DIN], F32, kind="ExternalInput")
    lnqb_d = nc.dram_tensor("lnqb", [1, DIN], F32, kind="ExternalInput")
    lnpg_d = nc.dram_tensor("lnpg", [1, HDIM], F32, kind="ExternalInput")
    lnpb_d = nc.dram_tensor("lnpb", [1, HDIM], F32, kind="ExternalInput")
    y_d = nc.dram_tensor("y", [N, DIN], F32, kind="ExternalOutput")

    with tile.TileContext(nc) as tc, ExitStack() as top:
        const = top.enter_context(tc.tile_pool(name="const", bufs=1))
        ones_r = const.tile([1, 128], F32, tag="ones_r")
        nc.vector.memset(ones_r[:], 1.0)
        ones_rb = const.tile([1, 128], BF16, tag="ones_rb")
        nc.vector.memset(ones_rb[:], 1.0)
        cmT_q = const.tile([128, 4], F32, tag="cmT_q")
        cmT_p = const.tile([128, 2], F32, tag="cmT_p")

        big = top.enter_context(tc.tile_pool(name="big", bufs=1))
        # comp first: the C outer-product runs on the PE while the big
        # weight/x DMAs are still in flight
        comp_r = big.tile([NF, N], BF16, tag="comp_r")
        nc.sync.dma_start(comp_r[:], comp_d.ap())
        xt = [big.tile([128, N], BF16, tag=f"xt{c}", name=f"xt{c}")
              for c in range(4)]
        wqk = [big.tile([128, 2 * HDIM], BF16, tag=f"wqk{c}", name=f"wqk{c}")
               for c in range(4)]
        wv = [big.tile([128, HDIM], BF16, tag=f"wv{c}", name=f"wv{c}")
              for c in range(4)]
        wpm = [big.tile([128, DIN], BF16, tag=f"wpm{c}", name=f"wpm{c}")
               for c in range(2)]
        for c in range(4):
            nc.gpsimd.dma_start(xt[c][:], xtb_d.ap()[c * 128:(c + 1) * 128, :])
        for c in range(4):
            nc.gpsimd.dma_start(wqk[c][:], wqk_d.ap()[c * 128:(c + 1) * 128, :])
        for c in range(4):
            nc.gpsimd.dma_start(wv[c][:], wv_d.ap()[c * 128:(c + 1) * 128, :])
        wp_raw = [big.tile([128, DIN], BF16, tag=f"wpr{c}", name=f"wpr{c}")
                  for c in range(2)]
        for c in range(2):
            nc.gpsimd.dma_start(wp_raw[c][:], wp_d.ap()[c * 128:(c + 1) * 128, :])

        qkv = top.enter_context(tc.tile_pool(name="qkv", bufs=1))
        qkT = [qkv.tile([128, N], BF16, tag=f"qkT{j}", name=f"qkT{j}")
               for j in range(4)]  # j 0,1 = q head-pairs; 2,3 = k head-pairs
        vv = [qkv.tile([128, HH * (HD + 1)], BF16, tag=f"vv{m}", name=f"vv{m}")
              for m in range(8)]
        Ct = qkv.tile([128, 8 * N], BF16, tag="Ct")  # C[nc*128+p, m]
        ymT = [qkv.tile([128, N], BF16, tag=f"ymT{c}", name=f"ymT{c}")
               for c in range(2)]

        # PSUM: exactly 8 banks
        psS = top.enter_context(tc.tile_pool(name="psS", bufs=2, space="PSUM"))
        psY = top.enter_context(tc.tile_pool(name="psY", bufs=1, space="PSUM"))
        psW = top.enter_context(tc.tile_pool(name="psW", bufs=1, space="PSUM"))

        # ---------- phase C: compatibility outer product (during DMA) ------
        for nch in range(8):
            ps = psS.tile([128, N], F32, tag="ps_s", name="ps_c")
            for half in range(2):
                nc.tensor.matmul(ps[:, half * 512:(half + 1) * 512],
                                 comp_r[:, nch * 128:(nch + 1) * 128],
                                 comp_r[:, half * 512:(half + 1) * 512],
                                 start=True, stop=True)
            # PSUM->SBUF moves split across ACT/DVE; both idle this early
            if nch % 2 == 0:
                nc.vector.tensor_copy(Ct[:, nch * N:(nch + 1) * N], ps[:])
            else:
                nc.scalar.copy(Ct[:, nch * N:(nch + 1) * N], ps[:])

        # ---------- phase A: modulation vectors ----------
        with tc.tile_pool(name="smA", bufs=1) as smA:
            wct_t = smA.tile([128, DIN], F32, tag="wct")
            nc.sync.dma_start(wct_t[:], wct_d.ap())
            codef_t = smA.tile([128, 1], F32, tag="codef")
            nc.sync.dma_start(codef_t[:], codef_d.ap())
            lnt = {}
            for nm, d in (("qg", lnqg_d), ("qb", lnqb_d)):
                lnt[nm] = smA.tile([1, DIN], F32, tag=f"ln{nm}", name=f"ln{nm}")
                nc.sync.dma_start(lnt[nm][:], d.ap())
            for nm, d in (("pg", lnpg_d), ("pb", lnpb_d)):
                lnt[nm] = smA.tile([1, HDIM], F32, tag=f"ln{nm}", name=f"ln{nm}")
                nc.sync.dma_start(lnt[nm][:], d.ap())

            cm0_ps = psW.tile([1, DIN], F32, tag="psw", name="cm0ps")
            nc.tensor.matmul(cm0_ps[:], codef_t[:], wct_t[:], start=True,
                             stop=True)
            cm0 = smA.tile([1, DIN], F32, tag="cm0")
            nc.vector.tensor_copy(cm0[:], cm0_ps[:])

            st = smA.tile([1, 1], F32, tag="st")
            nc.vector.tensor_reduce(st[:], cm0[:], mybir.AxisListType.X, OP.add)
            mu = smA.tile([1, 1], F32, tag="mu")
            nc.vector.tensor_scalar_mul(mu[:], st[:], 1.0 / DIN)
            sq = smA.tile([1, DIN], F32, tag="sq")
            vacc = smA.tile([1, 1], F32, tag="vacc")
            nc.vector.scalar_tensor_tensor(sq[:], cm0[:], mu[:], cm0[:],
                                           OP.subtract, OP.mult,
                                           accum_out=vacc[:])
            ve = smA.tile([1, 1], F32, tag="ve")
            nc.vector.tensor_scalar(ve[:], vacc[:], 1.0 / DIN, 1e-5,
                                    OP.mult, OP.add)
            sd = smA.tile([1, 1], F32, tag="sd")
            nc.scalar.activation(sd[:], ve[:], AT.Sqrt)
            rstd = smA.tile([1, 1], F32, tag="rstd")
            nc.vector.reciprocal(rstd[:], sd[:])

            def layer_norm(pref, g, b, dim):
                rg = smA.tile([1, dim], F32, tag=f"{pref}rg", name=f"{pref}rg")
                nc.vector.tensor_scalar_mul(rg[:], g[:], rstd[:])
                cx = smA.tile([1, dim], F32, tag=f"{pref}cx", name=f"{pref}cx")
                nc.vector.scalar_tensor_tensor(cx[:], cm0[:, 0:dim], mu[:],
                                               rg[:], OP.subtract, OP.mult)
                cm = smA.tile([1, dim], F32, tag=f"{pref}cm", name=f"{pref}cm")
                nc.vector.tensor_add(cm[:], cx[:], b[:])
                return cm

            cmq = layer_norm("q", lnt["qg"], lnt["qb"], DIN)
            cmp_ = layer_norm("p", lnt["pg"], lnt["pb"], HDIM)
            for c in range(4):
                tp = psW.tile([128, 1], F32, tag="psw", name="cmtp")
                nc.tensor.transpose(tp[:], cmq[:, c * 128:(c + 1) * 128],
                                    ones_r[0:1, 0:1])
                nc.vector.tensor_copy(cmT_q[:, c:c + 1], tp[:])
            for c in range(2):
                tp = psW.tile([128, 1], F32, tag="psw", name="cmtp")
                nc.tensor.transpose(tp[:], cmp_[:, c * 128:(c + 1) * 128],
                                    ones_r[0:1, 0:1])
                nc.vector.tensor_copy(cmT_p[:, c:c + 1], tp[:])
        for c in range(2):
            nc.vector.tensor_scalar_mul(wpm[c][:], wp_raw[c][:],
                                        cmT_p[:, c:c + 1])

        # ---------- phase B helpers ----------
        smB = top.enter_context(tc.tile_pool(name="smB", bufs=1))
        xm = [smB.tile([128, N], BF16, tag=f"xm{c}", name=f"xm{c}")
              for c in range(4)]
        for c in range(4):
            nc.vector.tensor_scalar_mul(xm[c][:], xt[c][:], cmT_q[:, c:c + 1])
        bqk_t = smB.tile([128, 4], F32, tag="bqk")
        for j in range(4):
            nc.sync.dma_start(bqk_t[:, j:j + 1],
                              bqk_d.ap()[j * 128:(j + 1) * 128])
        bv_raw = smB.tile([1, HDIM], F32, tag="bv_raw")
        nc.sync.dma_start(bv_raw[:], bv_d.ap())
        bvb = smB.tile([128, HDIM], F32, tag="bvb")
        nc.gpsimd.partition_broadcast(bvb[:], bv_raw[:], channels=128)
        bp_raw = const.tile([1, DIN], F32, tag="bp_raw")
        nc.sync.dma_start(bp_raw[:], bp_d.ap())
        bp_row = const.tile([1, DIN], BF16, tag="bp_row")
        nc.vector.tensor_copy(bp_row[:], bp_raw[:])

        def emit_qk(j):
            ps = psS.tile([128, N], F32, tag="ps_s", name="ps_qk")
            for half in range(2):
                for c in range(4):
                    nc.tensor.matmul(
                        ps[:, half * 512:(half + 1) * 512],
                        wqk[c][:, j * 128:(j + 1) * 128],
                        xm[c][:, half * 512:(half + 1) * 512],
                        start=(c == 0), stop=(c == 3))
            nc.vector.tensor_scalar_add(qkT[j][:], ps[:], bqk_t[:, j:j + 1])

        def emit_v(m):
            ps = psW.tile([128, HDIM], F32, tag="psw", name="ps_v")
            for c in range(4):
                nc.tensor.matmul(ps[:], xm[c][:, m * 128:(m + 1) * 128],
                                 wv[c][:], start=(c == 0), stop=(c == 3))
            v3 = vv[m][:].rearrange("p (h e) -> p h e", e=HD + 1)
            nc.vector.tensor_add(v3[:, :, 0:HD],
                                 ps[:].rearrange("p (h e) -> p h e", e=HD),
                                 bvb[:].rearrange("p (h e) -> p h e", e=HD))
            nc.vector.memset(v3[:, :, HD:HD + 1], 1.0)

        # ---------- phase D ----------
        smE1 = top.enter_context(tc.tile_pool(name="smE1", bufs=2))
        smT2 = top.enter_context(tc.tile_pool(name="smT2", bufs=2))
        smTT = top.enter_context(tc.tile_pool(name="smTT", bufs=2))
        smZ = top.enter_context(tc.tile_pool(name="smZ", bufs=2))
        smS = top.enter_context(tc.tile_pool(name="smS", bufs=2))
        smE = top.enter_context(tc.tile_pool(name="smE", bufs=3))
        state = {}

        def d1_scores(h, chunks):
            """Scores + exp1(+s accum) for the given query chunks."""
            qj, qo = h // 2, (h % 2) * 64
            if chunks[0] == 0:
                state[h] = dict(
                    e1=smE1.tile([128, 8 * N], BF16, tag="e1", name="e1"),
                    t2=smT2.tile([128, 8 * N], BF16, tag="t2", name="t2"),
                    srs=smS.tile([128, 16], F32, tag="srs", name="srs"))
            e1, srs = state[h]["e1"], state[h]["srs"]
            for nch in chunks:
                ps = psS.tile([128, N], F32, tag="ps_s", name="ps_s")
                for half in range(2):
                    nc.tensor.matmul(
                        ps[:, half * 512:(half + 1) * 512],
                        qkT[qj][qo:qo + 64, nch * 128:(nch + 1) * 128],
                        qkT[2 + qj][qo:qo + 64, half * 512:(half + 1) * 512],
                        start=True, stop=True)
                nc.scalar.activation(e1[:, nch * N:(nch + 1) * N], ps[:],
                                     AT.Exp, scale=SCALE,
                                     accum_out=srs[:, nch:nch + 1])

        def d1_norm(h):
            """Batched 1/s, then t2 = (e1 * 1/s) * C per chunk."""
            st = state[h]
            e1, t2, srs = st["e1"], st["t2"], st["srs"]
            nc.vector.reciprocal(srs[:, 8:16], srs[:, 0:8])
            for nch in range(8):
                nc.vector.tensor_scalar_mul(t2[:, nch * N:(nch + 1) * N],
                                            e1[:, nch * N:(nch + 1) * N],
                                            srs[:, 8 + nch:9 + nch])
                nc.vector.tensor_mul(t2[:, nch * N:(nch + 1) * N],
                                     t2[:, nch * N:(nch + 1) * N],
                                     Ct[:, nch * N:(nch + 1) * N])

        def d2(h, spread=False):
            """exp2 (2 big in-place passes) + XBAR transposes."""
            t2 = state[h]["t2"]
            e2T = smTT.tile([128, 8 * N], BF16, tag="e2T", name="e2T")
            e2T3 = e2T[:].rearrange("p (mc q) -> p mc q", q=N)
            state[h]["e2T"] = e2T
            for ph in range(2):
                nc.scalar.activation(t2[:, ph * 4 * N:(ph + 1) * 4 * N],
                                     t2[:, ph * 4 * N:(ph + 1) * 4 * N],
                                     AT.Exp)
                for g in range(4 * ph, 4 * ph + 4):
                    eng = nc.scalar if (spread and g % 2) else nc.sync
                    eng.dma_start_transpose(
                        e2T3[:, :, g * 128:(g + 1) * 128],
                        t2[:, g * N:(g + 1) * N])

        def d3(h):
            """PV + z2 normalization into ymT (bf16)."""
            e2T = state.pop(h)["e2T"]
            ypv = psY.tile([HD + 1, N], F32, tag="ypv", name="ypv")
            for mc in range(8):
                for half in range(2):
                    nc.tensor.matmul(
                        ypv[:, half * 512:(half + 1) * 512],
                        vv[mc][:, h * (HD + 1):(h + 1) * (HD + 1)],
                        e2T[:, mc * N + half * 512:mc * N + (half + 1) * 512],
                        start=(mc == 0), stop=(mc == 7))
            zr = smZ.tile([1, N], BF16, tag="zr", name="zr")
            with nc.allow_low_precision(reason="1/z2 in bf16; z2 is O(1e3)"):
                nc.vector.reciprocal(zr[:], ypv[HD:HD + 1, :])
            zbs = smZ.tile([64, N], BF16, tag="zbs", name="zbs")
            nc.gpsimd.partition_broadcast(zbs[:], zr[:], channels=64)
            nc.vector.tensor_mul(ymT[h // 2][(h % 2) * 64:(h % 2) * 64 + 64, :],
                                 ypv[0:HD, :], zbs[:])

        def proj(nb):
            ps = psS.tile([128, DIN], F32, tag="ps_s", name="ps_e")
            nc.tensor.matmul(ps[:], ones_rb[:], bp_row[:], start=True,
                             stop=False)
            for c in range(2):
                nc.tensor.matmul(ps[:], ymT[c][:, nb * 128:(nb + 1) * 128],
                                 wpm[c][:], start=False, stop=(c == 1))
            yo = smE.tile([128, DIN], F32, tag="yo", name="yo")
            if nb % 2 == 0:
                nc.vector.tensor_copy(yo[:], ps[:])
            else:
                nc.scalar.copy(yo[:], ps[:])
            nc.sync.dma_start(y_d.ap()[nb * 128:(nb + 1) * 128, :], yo[:])

        # -------- software-pipelined emission --------
        emit_qk(0)
        emit_qk(2)
        d1_scores(0, range(0, 4))
        emit_qk(1)
        d1_scores(0, range(4, 8))
        emit_qk(3)
        d1_norm(0)
        d1_scores(1, range(0, 4))
        emit_v(0); emit_v(1); emit_v(2); emit_v(3)
        d1_scores(1, range(4, 8))
        emit_v(4); emit_v(5); emit_v(6); emit_v(7)
        d2(0)
        d1_norm(1)
        d1_scores(2, range(0, 8))
        d2(1)
        d1_norm(2)
        d3(0)
        d1_scores(3, range(0, 8))
        d2(2)
        d3(1)
        d1_norm(3)
        d2(3, spread=True)
        d3(2)
        d3(3)
        for nb in range(8):
            proj(nb)

    nc.compile()
    return nc


def make_in_maps(x, compatibility, code, w_c, W_qkv, b_qkv, W_proj, b_proj,
                 ln_qkv_g, ln_qkv_b, ln_proj_g, ln_proj_b):
    bf = ml_dtypes.bfloat16
    x = np.asarray(x, np.float32)
    compatibility = np.asarray(compatibility, np.float32)
    code = np.asarray(code, np.float32)
    w_c = np.asarray(w_c, np.float32)
    W_qkv = np.asarray(W_qkv, np.float32)
    b_qkv = np.asarray(b_qkv, np.float32)
    W_proj = np.asarray(W_proj, np.float32)
    b_proj = np.asarray(b_proj, np.float32)
    ln_qkv_g = np.asarray(ln_qkv_g, np.float32)
    ln_qkv_b = np.asarray(ln_qkv_b, np.float32)
    ln_proj_g = np.asarray(ln_proj_g, np.float32)
    ln_proj_b = np.asarray(ln_proj_b, np.float32)

    xT = x[0].T  # [din, n]
    comp_bf = compatibility[0].astype(bf)
    in_maps = []
    for core in range(N_CORES):
        f, hh = core // 2, core % 2
        d0 = hh * HDIM
        # din permutation: this core's proj-input dims first
        perm = np.r_[d0:d0 + HDIM,
                     np.setdiff1d(np.arange(DIN), np.arange(d0, d0 + HDIM))]
        qrows = np.r_[d0:d0 + HDIM]          # q out-dims for heads hh*4..
        krows = np.r_[DIN + d0:DIN + d0 + HDIM]
        vrows = np.r_[2 * DIN + d0:2 * DIN + d0 + HDIM]
        in_maps.append(dict(
            xtb=np.ascontiguousarray(xT[perm, :]).astype(bf),
            wqk=np.ascontiguousarray(
                W_qkv[np.r_[qrows, krows], :][:, perm].T).astype(bf),
            wv=np.ascontiguousarray(W_qkv[vrows, :][:, perm].T).astype(bf),
            wp=np.ascontiguousarray(W_proj.T[d0:d0 + HDIM, :]).astype(bf),
            comp=comp_bf,
            codef=np.ascontiguousarray(code[:, f:f + 1]),
            wct=np.ascontiguousarray(w_c.T[:, perm]),
            bqk=np.ascontiguousarray(b_qkv[np.r_[qrows, krows]]),
            bv=np.ascontiguousarray(b_qkv[vrows]).reshape(1, HDIM),
            bp=(b_proj * 0.5).reshape(1, DIN),
            lnqg=np.ascontiguousarray(ln_qkv_g[perm]).reshape(1, DIN),
            lnqb=np.ascontiguousarray(ln_qkv_b[perm]).reshape(1, DIN),
            lnpg=np.ascontiguousarray(
                ln_proj_g[d0:d0 + HDIM]).reshape(1, HDIM),
            lnpb=np.ascontiguousarray(
                ln_proj_b[d0:d0 + HDIM]).reshape(1, HDIM),
        ))
    return in_maps


def kernel(**inputs) -> np.ndarray:
    from concourse.bass_utils import run_bass_kernel_spmd
    if "nc" not in _CACHE:
        _CACHE["nc"] = build_nc()
    nc = _CACHE["nc"]
    in_maps = make_in_maps(**inputs)
    res = run_bass_kernel_spmd(nc, in_maps, core_ids=list(range(N_CORES)))
    out = np.zeros((1, NF, N, DIN), np.float32)
    for core in range(N_CORES):
        f = core // 2
        out[0, f] += np.asarray(res.results[core]["y"], np.float32)
    return out
